# revision 1
# baseline (speedup 1.0000x reference)
"""Trainium2 Bass kernel for nn_CANLayer (gnn_message_passing).

Math: softmax over a singleton axis makes the attention weights identically
1.0, so each conv is a plain sparse matmul:
    out = sigmoid(A_d @ x @ Wd + A_u @ x @ Wu + (1+eps) x @ Wi) ; out *= elu(out @ a)

Strategy (8 cores, SPMD single program, per-core data):
  - shard targets: core k owns rows [k*12500, (k+1)*12500); x_1 replicated
  - per core+Laplacian: edges tgt-sorted, grouped into windows of 500
    targets; 128-message chunks; each chunk gathered from x_1 by row via
    indirect DMA (128 rows / instruction) and scattered into a PSUM window
    via a PE matmul against an on-chip-built selection matrix
    S[slot, t] = val * (rel[slot] == t)
  - y^T accumulated in SBUF; dense epilogue (W matmuls, sigmoid, elu gate)
"""
import numpy as np

import concourse.bacc as bacc
import concourse.bass as bass
import concourse.mybir as mybir
import concourse.tile as tile
from concourse.bass import ds, ts
from concourse.bass_utils import run_bass_kernel_spmd

N = 100000
C = 64
NCORES = 8
TPC = 12500
WIN = 500
NW = TPC // WIN          # 25
EPS = 1e-5
PAD_IDX = 1 << 24        # OOB sentinel (skipped via bounds_check)

LAST_EXEC_NS = None

_frontend_cache = {}


def _preprocess(indices, values):
    """Per (core, lap): chunked tgt-sorted streams.

    Returns per core: list over windows of (idx[int32 m], val[f32 m], rel[f32 m])
    """
    tgt = np.asarray(indices[0], np.int64)
    src = np.asarray(indices[1], np.int64)
    val = np.asarray(values, np.float32)
    out = []
    for k in range(NCORES):
        base = k * TPC
        sel = (tgt >= base) & (tgt < base + TPC)
        tl = tgt[sel] - base
        s = src[sel]
        v = val[sel]
        order = np.argsort(tl, kind="stable")
        tl, s, v = tl[order], s[order], v[order]
        w = tl // WIN
        # split per window
        cuts = np.searchsorted(w, np.arange(1, NW))
        idx_w = np.split(s, cuts)
        rel_w = np.split(tl - w * WIN, cuts)
        val_w = np.split(v, cuts)
        out.append((idx_w, val_w, rel_w))
    return out


def _build_program(CH):
    """CH[lap][w] = chunk count. Returns (nc, meta) with SPMD program."""
    nc = bacc.Bacc("TRN2", target_bir_lowering=False, debug=False)
    f32 = mybir.dt.float32
    i32 = mybir.dt.int32

    nch = [int(sum(CH[L])) for L in range(2)]
    x = nc.dram_tensor("x", [N, C], f32, kind="ExternalInput")
    xT = nc.dram_tensor("xT", [C, TPC], f32, kind="ExternalInput")
    idx_d = [nc.dram_tensor(f"idx{L}", [128, nch[L]], i32, kind="ExternalInput") for L in range(2)]
    val_d = [nc.dram_tensor(f"val{L}", [128, nch[L]], f32, kind="ExternalInput") for L in range(2)]
    rel_d = [nc.dram_tensor(f"rel{L}", [128, nch[L]], f32, kind="ExternalInput") for L in range(2)]
    wts = nc.dram_tensor("wts", [C, 3 * C + 1], f32, kind="ExternalInput")  # Wd|Wu|Wi*(1+eps)|att
    iota_d = nc.dram_tensor("iota", [128, WIN], f32, kind="ExternalInput")
    ident_d = nc.dram_tensor("ident", [128, 128], f32, kind="ExternalInput")
    out_d = nc.dram_tensor("out", [TPC, C], f32, kind="ExternalOutput")
    ybuf = [nc.dram_tensor(f"ybuf{L}", [C, TPC], f32) for L in range(2)]
    sbuf_d = nc.dram_tensor("sbuf_d", [C + 1, TPC], f32)

    NT128 = TPC // 128 + (1 if TPC % 128 else 0)  # 98 blocks of <=128 rows

    with tile.TileContext(nc) as tc:
        with (
            tc.tile_pool(name="const", bufs=1) as constp,
            tc.tile_pool(name="meta", bufs=1) as metap,
            tc.tile_pool(name="msg", bufs=12) as msgp,
            tc.tile_pool(name="st", bufs=6) as stp,
            tc.tile_pool(name="ypsum", bufs=3, space="PSUM") as ypsum,
            tc.tile_pool(name="ysb", bufs=1) as ysbp,
            tc.tile_pool(name="r2", bufs=2, space="PSUM") as r2psum,
            tc.tile_pool(name="gp", bufs=1, space="PSUM") as gpsum,
            tc.tile_pool(name="tp", bufs=2, space="PSUM") as tpsum,
            tc.tile_pool(name="ep", bufs=4) as epool,
        ):
            iota_t = constp.tile([128, WIN], f32)
            nc.sync.dma_start(out=iota_t[:], in_=iota_d[:])
            ident_t = constp.tile([128, 128], f32)
            nc.sync.dma_start(out=ident_t[:], in_=ident_d[:])
            wts_t = constp.tile([C, 3 * C + 1], f32)
            nc.sync.dma_start(out=wts_t[:], in_=wts[:])

            idx_t = [metap.tile([128, nch[L]], i32, tag=f"idx{L}", name=f"idx_t{L}") for L in range(2)]
            val_t = [metap.tile([128, nch[L]], f32, tag=f"val{L}", name=f"val_t{L}") for L in range(2)]
            rel_t = [metap.tile([128, nch[L]], f32, tag=f"rel{L}", name=f"rel_t{L}") for L in range(2)]
            for L in range(2):
                nc.sync.dma_start(out=idx_t[L][:], in_=idx_d[L][:])
                nc.sync.dma_start(out=val_t[L][:], in_=val_d[L][:])
                nc.sync.dma_start(out=rel_t[L][:], in_=rel_d[L][:])

            # zero the msg pool slots once: pad slots are skipped by the
            # gather's bounds check and would otherwise read stale SBUF
            for _ in range(12):
                mwarm = msgp.tile([128, C], f32, tag="msg")
                nc.vector.memset(mwarm[:], 0.0)

            # ---- scatter phase ----
            for L in range(2):
                c0 = 0
                for w in range(NW):
                    nchw = CH[L][w]
                    wn = min(WIN, TPC - w * WIN)
                    ps = ypsum.tile([C, WIN], f32, tag="yps")
                    for i in range(nchw):
                        c = c0 + i
                        msg = msgp.tile([128, C], f32, tag="msg")
                        nc.gpsimd.indirect_dma_start(
                            out=msg[:],
                            out_offset=None,
                            in_=x[:],
                            in_offset=bass.IndirectOffsetOnAxis(ap=idx_t[L][:, c:c + 1], axis=0),
                            bounds_check=N - 1,
                            oob_is_err=False,
                        )
                        st = stp.tile([128, WIN], f32, tag="st")
                        nc.vector.tensor_scalar(
                            out=st[:],
                            in0=iota_t[:],
                            scalar1=rel_t[L][:, c:c + 1],
                            scalar2=val_t[L][:, c:c + 1],
                            op0=mybir.AluOpType.is_equal,
                            op1=mybir.AluOpType.mult,
                        )
                        nc.tensor.matmul(
                            out=ps[:],
                            lhsT=msg[:],
                            rhs=st[:],
                            start=(i == 0),
                            stop=(i == nchw - 1),
                        )
                    ytmp = ysbp.tile([C, WIN], f32, tag="ytmp")
                    nc.scalar.copy(out=ytmp[:, :wn], in_=ps[:, :wn])
                    nc.sync.dma_start(out=ybuf[L][:, w * WIN:w * WIN + wn], in_=ytmp[:, :wn])
                    c0 += nchw

            # ---- dense epilogue ----
            for w in range(NW):
                wn = min(WIN, TPC - w * WIN)
                sl = slice(w * WIN, w * WIN + wn)
                y0w = ysbp.tile([C, WIN], f32, tag="y0w")
                y1w = ysbp.tile([C, WIN], f32, tag="y1w")
                xTw = ysbp.tile([C, WIN], f32, tag="xTw")
                nc.sync.dma_start(out=y0w[:, :wn], in_=ybuf[0][:, sl])
                nc.sync.dma_start(out=y1w[:, :wn], in_=ybuf[1][:, sl])
                nc.sync.dma_start(out=xTw[:, :wn], in_=xT[:, sl])
                r = r2psum.tile([C, WIN], f32, tag="r")
                nc.tensor.matmul(out=r[:, :wn], lhsT=wts_t[:, 0:C], rhs=y0w[:, :wn], start=True, stop=False)
                nc.tensor.matmul(out=r[:, :wn], lhsT=wts_t[:, C:2 * C], rhs=y1w[:, :wn], start=False, stop=False)
                nc.tensor.matmul(out=r[:, :wn], lhsT=wts_t[:, 2 * C:3 * C], rhs=xTw[:, :wn], start=False, stop=True)
                s_sb = ysbp.tile([C + 1, WIN], f32, tag="s_sb")
                nc.scalar.activation(out=s_sb[0:C, :wn], in_=r[:, :wn], func=mybir.ActivationFunctionType.Sigmoid)
                g = gpsum.tile([1, WIN], f32, tag="g")
                nc.tensor.matmul(out=g[:, :wn], lhsT=wts_t[:, 3 * C:3 * C + 1], rhs=s_sb[0:C, :wn], start=True, stop=True)
                # elu(g) = max(g,0) + exp(min(g,0)) - 1
                t1 = epool.tile([1, WIN], f32, tag="t1")
                t2 = epool.tile([1, WIN], f32, tag="t2")
                nc.vector.tensor_scalar_max(out=t1[:, :wn], in0=g[:, :wn], scalar1=0.0)
                nc.vector.tensor_scalar_min(out=t2[:, :wn], in0=g[:, :wn], scalar1=0.0)
                nc.scalar.activation(out=t2[:, :wn], in_=t2[:, :wn], func=mybir.ActivationFunctionType.Exp)
                nc.vector.tensor_tensor(out=t1[:, :wn], in0=t1[:, :wn], in1=t2[:, :wn], op=mybir.AluOpType.add)
                nc.vector.tensor_scalar_add(out=s_sb[C:C + 1, :wn], in0=t1[:, :wn], scalar1=-1.0)
                nc.sync.dma_start(out=sbuf_d[:, sl], in_=s_sb[:, :wn])

            # ---- transpose + gate + store ----
            for tb in range(NT128):
                r0 = tb * 128
                rn = min(128, TPC - r0)
                scol = epool.tile([C + 1, 128], f32, tag="scol")
                nc.sync.dma_start(out=scol[:, :rn], in_=sbuf_d[:, r0:r0 + rn])
                pt = tpsum.tile([128, C + 1], f32, tag="pt")
                nc.tensor.transpose(
                    out=pt[:rn, :],
                    in_=scol[:, :rn],
                    identity=ident_t[:C + 1, :C + 1],
                )
                gate = epool.tile([128, 1], f32, tag="gate")
                nc.scalar.copy(out=gate[:rn, :], in_=pt[:rn, C:C + 1])
                ot = epool.tile([128, C], f32, tag="ot")
                nc.vector.tensor_scalar(
                    out=ot[:rn, :],
                    in0=pt[:rn, 0:C],
                    scalar1=gate[:rn, :],
                    scalar2=None,
                    op0=mybir.AluOpType.mult,
                )
                nc.sync.dma_start(out=out_d[r0:r0 + rn, :], in_=ot[:rn, :])
    nc.compile()
    return nc


def kernel(x_1, down_indices, down_values, up_indices, up_values,
           W_down, W_up, W_id, att_down, att_up, att_layer):
    global LAST_EXEC_NS
    x_1 = np.ascontiguousarray(np.asarray(x_1, np.float32))

    pre = [_preprocess(down_indices, down_values), _preprocess(up_indices, up_values)]

    # chunk counts, shared across cores (SPMD)
    CH = []
    for L in range(2):
        ch = []
        for w in range(NW):
            m = max(len(pre[L][k][0][w]) for k in range(NCORES))
            ch.append(max(1, (m + 127) // 128))
        CH.append(ch)
    nch = [int(sum(CH[L])) for L in range(2)]

    # per-core metadata arrays
    in_maps = []
    iota = np.broadcast_to(np.arange(WIN, dtype=np.float32), (128, WIN)).copy()
    ident = np.eye(128, dtype=np.float32)
    wts = np.concatenate(
        [np.asarray(W_down, np.float32), np.asarray(W_up, np.float32),
         (1.0 + EPS) * np.asarray(W_id, np.float32), np.asarray(att_layer, np.float32)],
        axis=1,
    )
    for k in range(NCORES):
        m = {"x": x_1, "xT": np.ascontiguousarray(x_1[k * TPC:(k + 1) * TPC].T),
             "wts": wts, "iota": iota, "ident": ident}
        for L in range(2):
            S = nch[L] * 128
            idx = np.full(S, PAD_IDX, np.int32)
            val = np.zeros(S, np.float32)
            rel = np.zeros(S, np.float32)
            off = 0
            idx_w, val_w, rel_w = pre[L][k]
            for w in range(NW):
                n = len(idx_w[w])
                idx[off:off + n] = idx_w[w]
                val[off:off + n] = val_w[w]
                rel[off:off + n] = rel_w[w]
                off += CH[L][w] * 128
            m[f"idx{L}"] = idx.reshape(-1, 128).T.copy()
            m[f"val{L}"] = val.reshape(-1, 128).T.copy()
            m[f"rel{L}"] = rel.reshape(-1, 128).T.copy()
        in_maps.append(m)

    key = (tuple(CH[0]), tuple(CH[1]))
    if key not in _frontend_cache:
        _frontend_cache.clear()
        _frontend_cache[key] = _build_program(CH)
    nc = _frontend_cache[key]

    res = run_bass_kernel_spmd(nc, in_maps, core_ids=list(range(NCORES)), trace=True)
    LAST_EXEC_NS = res.exec_time_ns
    out = np.concatenate([res.results[k]["out"] for k in range(NCORES)], axis=0)
    return out.astype(np.float32)

